# revision 8
# baseline (speedup 1.0000x reference)
"""NetVLAD Trainium2 kernel — 8-core SPMD Bass/Tile implementation.

Reference computation (N=32, C=512, T=300, R=4, K=64, OUT=1024):
  xf = x.reshape(N, C, T*R)
  logits = einsum('kc,nct->nkt', conv_w, xf); assign = softmax_k(logits)
  am = assign * mask;  vlad = einsum('nkt,nct->nkc', am, xf) - am.sum(t)*centroids
  vlad = l2norm_c(vlad).reshape(N, K*C); vlad = l2norm(vlad)
  y = vlad @ red_w.T;  out = layernorm(y) * ln_w + ln_b

Sharding: stage 1 data-parallel over N (4 samples/core, fp32);
bf16 AllGather of the per-sample normalized descriptors (global-norm folded in);
stage 2 output-parallel over OUT (128 cols/core) with bf16 weights;
AllReduce of LayerNorm partial sums; LN epilogue on device.

Host passes x in both c-major and t-major layouts so every matmul sees its
operands in natural orientation (no on-device transposes).
"""
import numpy as np
import ml_dtypes

import concourse.bass as bass
import concourse.bacc as bacc
import concourse.tile as tile
import concourse.mybir as mybir
from concourse.bass_utils import run_bass_kernel_spmd

NCORES = 8
N, C, T, R, K, OUT = 32, 512, 300, 4, 64, 1024
TR = T * R            # 1200
TRP = 1280            # padded to 10 chunks of 128 (pads masked out)
NJ = TRP // 128       # 10
NL = N // NCORES      # 4 samples per core
CC = C // 128         # 4 c-chunks
OL = OUT // NCORES    # 128 output cols per core
F32 = mybir.dt.float32
BF16 = mybir.dt.bfloat16
AX = mybir.AxisListType
AF = mybir.ActivationFunctionType

_CACHE = {}


def _build(debug=False):
    nc = bacc.Bacc("TRN2", target_bir_lowering=False, debug=False,
                   num_devices=NCORES)
    xc = nc.dram_tensor("xc", [NL, C, TRP], F32, kind="ExternalInput")
    xt = nc.dram_tensor("xt", [NL, TRP, C], F32, kind="ExternalInput")
    mf = nc.dram_tensor("mf", [NL, 128, NJ], F32, kind="ExternalInput")
    cwT_d = nc.dram_tensor("cwT", [C, K], F32, kind="ExternalInput")
    cent_d = nc.dram_tensor("cent", [K, C], F32, kind="ExternalInput")
    id64_d = nc.dram_tensor("id64", [K, K], F32, kind="ExternalInput")
    w2_d = nc.dram_tensor("w2", [CC, 128, K * 128], BF16, kind="ExternalInput")
    lnw_d = nc.dram_tensor("lnw", [OL, 1], F32, kind="ExternalInput")
    lnb_d = nc.dram_tensor("lnb", [OL, 1], F32, kind="ExternalInput")
    yT_d = nc.dram_tensor("yT", [OL, N], F32, kind="ExternalOutput")
    if debug:
        dbg_vs_d = nc.dram_tensor("dbg_vs", [CC * 128, NL * K], BF16,
                                  kind="ExternalOutput")
        dbg_ys_d = nc.dram_tensor("dbg_ys", [OL, N], F32,
                                  kind="ExternalOutput")
        dbg_am_d = nc.dram_tensor("dbg_am", [TRP, K], F32,
                                  kind="ExternalOutput")
        dbg_vt_d = nc.dram_tensor("dbg_vt", [128, CC * K], F32,
                                  kind="ExternalOutput")
        dbg_sc_d = nc.dram_tensor("dbg_sc", [2, K], F32,
                                  kind="ExternalOutput")
    rg = [list(range(NCORES))]

    with tile.TileContext(nc) as tc:
        with tc.tile_pool(name="constp", bufs=1) as constp, \
             tc.tile_pool(name="xcp", bufs=8) as xcp, \
             tc.tile_pool(name="xtp", bufs=2 * NJ) as xtp, \
             tc.tile_pool(name="amp", bufs=2 * NJ) as amp, \
             tc.tile_pool(name="smp", bufs=4) as smp, \
             tc.tile_pool(name="vsp", bufs=2) as vsp, \
             tc.tile_pool(name="w2p", bufs=1) as w2p, \
             tc.tile_pool(name="vgp", bufs=1) as vgp, \
             tc.tile_pool(name="psA", bufs=2, space="PSUM") as psA, \
             tc.tile_pool(name="psV", bufs=2, space="PSUM") as psV, \
             tc.tile_pool(name="psS", bufs=2, space="PSUM") as psS, \
             tc.tile_pool(name="psY", bufs=1, space="PSUM") as psY, \
             tc.tile_pool(name="dram", bufs=1, space="DRAM") as dram:

            # ---- constants ----
            ones = constp.tile([128, 1], F32, name="ones")
            nc.vector.memset(ones[:], 1.0)
            cwTt = []
            for cc in range(CC):
                cw = constp.tile([128, K], F32, name=f"cwTt{cc}")
                nc.sync.dma_start(out=cw[:], in_=cwT_d[cc * 128:(cc + 1) * 128, :])
                cwTt.append(cw)
            centt = []
            for cc in range(CC):
                ce = constp.tile([K, 128], F32, name=f"centt{cc}")
                nc.sync.dma_start(out=ce[:], in_=cent_d[:, cc * 128:(cc + 1) * 128])
                centt.append(ce)
            id64t = constp.tile([K, K], F32, name="id64t")
            nc.sync.dma_start(out=id64t[:], in_=id64_d[:])
            mtn = []
            for n in range(NL):
                mt = constp.tile([128, NJ], F32, name=f"mt{n}")
                nc.sync.dma_start(out=mt[:], in_=mf[n])
                mtn.append(mt)
            lnw_t = constp.tile([OL, 1], F32, name="lnw_t")
            nc.sync.dma_start(out=lnw_t[:], in_=lnw_d[:])
            lnb_t = constp.tile([OL, 1], F32, name="lnb_t")
            nc.sync.dma_start(out=lnb_t[:], in_=lnb_d[:])

            cc_in = dram.tile([CC * 128, NL * K], BF16, name="cc_in")

            # ---- stage 1: per-sample NetVLAD descriptor ----
            for n in range(NL):
                xq = []
                for cc in range(CC):
                    xcq = xcp.tile([128, TRP], F32, name="xcq")
                    nc.sync.dma_start(out=xcq[:],
                                      in_=xc[n, cc * 128:(cc + 1) * 128, :])
                    xq.append(xcq)
                xtq = []
                for jj in range(NJ):
                    xtt = xtp.tile([128, C], F32, name="xtt")
                    nc.sync.dma_start(out=xtt[:],
                                      in_=xt[n, jj * 128:(jj + 1) * 128, :])
                    xtq.append(xtt)

                # logits (TR-major) + masked softmax over K
                amT = []
                for jj in range(NJ):
                    lt = psA.tile([128, K], F32, name="lt")
                    for cc in range(CC):
                        nc.tensor.matmul(
                            lt[:], lhsT=xq[cc][:, jj * 128:(jj + 1) * 128],
                            rhs=cwTt[cc][:],
                            start=(cc == 0), stop=(cc == CC - 1))
                    negmax = smp.tile([128, 1], F32, name="negmax")
                    nc.vector.reduce_max(negmax[:], lt[:], axis=AX.X, negate=True)
                    et = smp.tile([128, K], F32, name="et")
                    s = smp.tile([128, 1], F32, name="s")
                    nc.scalar.activation(et[:], lt[:], AF.Exp,
                                         bias=negmax[:], scale=1.0,
                                         accum_out=s[:])
                    rs = smp.tile([128, 1], F32, name="rs")
                    nc.vector.reciprocal(rs[:], s[:])
                    rsm = smp.tile([128, 1], F32, name="rsm")
                    nc.vector.tensor_mul(rsm[:], rs[:], mtn[n][:, jj:jj + 1])
                    am = amp.tile([128, K], F32, name="am")
                    nc.vector.tensor_scalar_mul(am[:], et[:], rsm[:])
                    amT.append(am)
                    if debug and n == 0:
                        nc.sync.dma_start(
                            out=dbg_am_d[jj * 128:(jj + 1) * 128, :],
                            in_=am[:])

                # vlad^T accumulation: vt[c, cc*K+k] = sum_t x^T[t,c]*am[t,k]
                vt = psV.tile([128, CC * K], F32, name="vt")
                nc.vector.memset(vt[:], 0.0)
                for cc in range(CC):
                    for jj in range(NJ):
                        nc.tensor.matmul(
                            vt[:, cc * K:(cc + 1) * K],
                            lhsT=xtq[jj][:, cc * 128:(cc + 1) * 128],
                            rhs=amT[jj][:],
                            start=False, stop=False,
                            skip_group_check=True)
                # a[k] = sum_t am[t,k]
                a_ps = psS.tile([1, K], F32, name="a_ps", tag="small_ps")
                for jj in range(NJ):
                    nc.tensor.matmul(a_ps[:], lhsT=ones[:], rhs=amT[jj][:],
                                     start=(jj == 0), stop=(jj == NJ - 1))
                na_sb = smp.tile([1, K], F32, name="na_sb")
                nc.scalar.mul(na_sb[:], a_ps[:], -1.0)
                naB = smp.tile([K, K], F32, name="naB")
                nc.gpsimd.partition_broadcast(naB[:], na_sb[:])
                dga = smp.tile([K, K], F32, name="dga")
                nc.vector.tensor_mul(dga[:], id64t[:], naB[:])
                # vt -= a[k]*cent[k,c]  (cent.T @ diag(a) appended to the group)
                for cc in range(CC):
                    nc.tensor.matmul(vt[:, cc * K:(cc + 1) * K],
                                     lhsT=centt[cc][:], rhs=dga[:],
                                     start=False, stop=(cc == CC - 1),
                                     skip_group_check=True)

                # intra-norm over c + global norm, fused into one scale
                if debug and n == 0:
                    vt_sb = smp.tile([128, CC * K], F32, name="vt_sb")
                    nc.scalar.copy(vt_sb[:], vt[:])
                    nc.sync.dma_start(out=dbg_vt_d[:], in_=vt_sb[:])
                vsq = smp.tile([128, CC * K], F32, name="vsq")
                nc.scalar.square(vsq[:], vt[:])
                ssq_ps = psS.tile([1, CC * K], F32, name="ssq_ps", tag="small_ps")
                nc.tensor.matmul(ssq_ps[:], lhsT=ones[:], rhs=vsq[:],
                                 start=True, stop=True)
                ssq = smp.tile([1, K], F32, name="ssq")
                nc.vector.reduce_sum(
                    ssq[:],
                    ssq_ps.rearrange("p (cc k) -> p k cc", cc=CC)[:],
                    axis=AX.X)
                nrm = smp.tile([1, K], F32, name="nrm")
                nc.scalar.sqrt(nrm[:], ssq[:])
                nrmc = smp.tile([1, K], F32, name="nrmc")
                nc.vector.tensor_scalar_max(nrmc[:], nrm[:], 1e-12)
                rn = smp.tile([1, K], F32, name="rn")
                nc.vector.reciprocal(rn[:], nrmc[:])
                t1 = smp.tile([1, K], F32, name="t1")
                nc.vector.tensor_mul(t1[:], ssq[:], rn[:])
                t2 = smp.tile([1, K], F32, name="t2")
                nc.vector.tensor_mul(t2[:], t1[:], rn[:])
                gsq = smp.tile([1, 1], F32, name="gsq")
                nc.vector.reduce_sum(gsq[:], t2[:], axis=AX.X)
                g = smp.tile([1, 1], F32, name="g")
                nc.scalar.sqrt(g[:], gsq[:])
                gc = smp.tile([1, 1], F32, name="gc")
                nc.vector.tensor_scalar_max(gc[:], g[:], 1e-12)
                gr = smp.tile([1, 1], F32, name="gr")
                nc.vector.reciprocal(gr[:], gc[:])
                rne = smp.tile([1, K], F32, name="rne")
                nc.vector.tensor_scalar_mul(rne[:], rn[:], gr[:])
                if debug and n == 0:
                    nc.sync.dma_start(out=dbg_sc_d[0:1, :], in_=ssq[:])
                    nc.sync.dma_start(out=dbg_sc_d[1:2, :], in_=rne[:])
                rnB = smp.tile([128, K], F32, name="rnB")
                nc.gpsimd.partition_broadcast(rnB[:], rne[:])
                vstar = vsp.tile([128, CC * K], BF16, name="vstar")
                for cc in range(CC):
                    nc.vector.tensor_mul(vstar[:, cc * K:(cc + 1) * K],
                                         vt[:, cc * K:(cc + 1) * K], rnB[:])
                nc.sync.dma_start(
                    out=cc_in.rearrange("(cc c) nk -> c cc nk",
                                        cc=CC)[:, :, n * K:(n + 1) * K],
                    in_=vstar.rearrange("c (cc k) -> c cc k", cc=CC)[:])

            if debug:
                nc.sync.dma_start(out=dbg_vs_d[:], in_=cc_in[:])

            # ---- AllGather descriptors ----
            cc_out = dram.tile([NCORES * CC * 128, NL * K], BF16,
                               addr_space="Shared", name="cc_out")
            nc.gpsimd.collective_compute(
                "AllGather", mybir.AluOpType.bypass, replica_groups=rg,
                ins=[cc_in.opt()], outs=[cc_out.opt()])

            # ---- stage 2: y^T[o, n] = sum_j v[n, j] * red_w[o, j] ----
            w2t = []
            for cc in range(CC):
                wt = w2p.tile([128, K * 128], BF16, name=f"w2t{cc}")
                nc.sync.dma_start(out=wt[:], in_=w2_d[cc])
                w2t.append(wt)
            vg = []
            for cc in range(CC):
                vgt = vgp.tile([128, NCORES * NL * K], BF16, name=f"vgt{cc}")
                nc.sync.dma_start(
                    out=vgt.rearrange("c (r nk) -> c r nk", r=NCORES)[:],
                    in_=cc_out.rearrange("(r cc c) nk -> cc c r nk",
                                         r=NCORES, cc=CC)[cc])
                vg.append(vgt)
            yT = psY.tile([OL, N], F32, name="yT")
            for cc in range(CC):
                vgv = vg[cc].rearrange("c (rn k) -> c k rn", k=K)
                for k in range(K):
                    nc.tensor.matmul(
                        yT[:], lhsT=w2t[cc][:, k * 128:(k + 1) * 128],
                        rhs=vgv[:, k],
                        start=(cc == 0 and k == 0),
                        stop=(cc == CC - 1 and k == K - 1))

            # ---- LayerNorm: stats over all OUT via AllReduce ----
            ys = smp.tile([OL, N], F32, name="ys")
            nc.scalar.copy(ys[:], yT[:])
            if debug:
                nc.sync.dma_start(out=dbg_ys_d[:], in_=ys[:])
            ysq = smp.tile([OL, N], F32, name="ysq")
            nc.vector.tensor_mul(ysq[:], ys[:], ys[:])
            st_ps = psS.tile([1, 2 * N], F32, name="st_ps", tag="small_ps")
            nc.vector.memset(st_ps[:], 0.0)
            nc.tensor.matmul(st_ps[:, :N], lhsT=ones[:], rhs=ys[:],
                             start=False, stop=True, skip_group_check=True)
            nc.tensor.matmul(st_ps[:, N:], lhsT=ones[:], rhs=ysq[:],
                             start=False, stop=True, skip_group_check=True)
            st_sb = smp.tile([1, 2 * N], F32, name="st_sb")
            nc.scalar.copy(st_sb[:], st_ps[:])
            ar_in = dram.tile([1, 2 * N], F32, name="ar_in")
            nc.sync.dma_start(out=ar_in[:], in_=st_sb[:])
            ar_out = dram.tile([1, 2 * N], F32, addr_space="Shared",
                               name="ar_out")
            nc.gpsimd.collective_compute(
                "AllReduce", mybir.AluOpType.add, replica_groups=rg,
                ins=[ar_in.opt()], outs=[ar_out.opt()])
            st = smp.tile([1, 2 * N], F32, name="st")
            nc.sync.dma_start(out=st[:], in_=ar_out[:])
            mu = smp.tile([1, N], F32, name="mu")
            nc.vector.tensor_scalar_mul(mu[:], st[:, :N], 1.0 / OUT)
            m2 = smp.tile([1, N], F32, name="m2")
            nc.vector.tensor_scalar_mul(m2[:], st[:, N:], 1.0 / OUT)
            musq = smp.tile([1, N], F32, name="musq")
            nc.vector.tensor_mul(musq[:], mu[:], mu[:])
            var = smp.tile([1, N], F32, name="var")
            nc.vector.tensor_sub(var[:], m2[:], musq[:])
            vare = smp.tile([1, N], F32, name="vare")
            nc.vector.tensor_scalar_add(vare[:], var[:], 1e-5)
            sd = smp.tile([1, N], F32, name="sd")
            nc.scalar.sqrt(sd[:], vare[:])
            rstd = smp.tile([1, N], F32, name="rstd")
            nc.vector.reciprocal(rstd[:], sd[:])
            muB = smp.tile([OL, N], F32, name="muB")
            nc.gpsimd.partition_broadcast(muB[:], mu[:])
            rstdB = smp.tile([OL, N], F32, name="rstdB")
            nc.gpsimd.partition_broadcast(rstdB[:], rstd[:])
            d1 = smp.tile([OL, N], F32, name="d1")
            nc.vector.tensor_sub(d1[:], ys[:], muB[:])
            d2 = smp.tile([OL, N], F32, name="d2")
            nc.vector.tensor_mul(d2[:], d1[:], rstdB[:])
            outv = smp.tile([OL, N], F32, name="outv")
            nc.scalar.activation(outv[:], d2[:], AF.Identity,
                                 bias=lnb_t[:], scale=lnw_t[:])
            nc.sync.dma_start(out=yT_d[:], in_=outv[:])

    nc.compile()
    return nc


def _prep_inputs(x, mask, centroids, conv_w, red_w, ln_w, ln_b):
    """Shard + lay out the full inputs per core."""
    x = np.asarray(x, dtype=np.float32)
    mask = np.asarray(mask)
    xf = x.reshape(N, C, TR)
    xc_full = np.zeros((N, C, TRP), dtype=np.float32)
    xc_full[:, :, :TR] = xf
    xt_full = np.ascontiguousarray(xc_full.transpose(0, 2, 1))
    m_e = np.zeros((N, TRP), dtype=np.float32)
    m_e[:, :TR] = np.repeat(np.asarray(mask, dtype=np.float32), R, axis=1)
    # mf[n, p, jj] = m_e[n, jj*128 + p]
    mf_full = np.ascontiguousarray(
        m_e.reshape(N, NJ, 128).transpose(0, 2, 1))
    cwT = np.ascontiguousarray(np.asarray(conv_w, dtype=np.float32).T)
    cent = np.ascontiguousarray(np.asarray(centroids, dtype=np.float32))
    id64 = np.eye(K, dtype=np.float32)
    rw = np.asarray(red_w, dtype=np.float32)
    lnw = np.asarray(ln_w, dtype=np.float32)
    lnb = np.asarray(ln_b, dtype=np.float32)

    in_maps = []
    for r in range(NCORES):
        ns = slice(r * NL, (r + 1) * NL)
        # w2[cc][c, k*128+o] = red_w[r*128+o, k*512+cc*128+c]
        wl = rw[r * OL:(r + 1) * OL].reshape(OL, K, CC, 128)  # [o,k,cc,c]
        w2 = np.stack(
            [np.ascontiguousarray(
                wl[:, :, cc, :].transpose(2, 1, 0).reshape(128, K * 128))
             for cc in range(CC)], axis=0).astype(ml_dtypes.bfloat16)
        in_maps.append({
            "xc": np.ascontiguousarray(xc_full[ns]),
            "xt": np.ascontiguousarray(xt_full[ns]),
            "mf": np.ascontiguousarray(mf_full[ns]),
            "cwT": cwT,
            "cent": cent,
            "id64": id64,
            "w2": w2,
            "lnw": np.ascontiguousarray(lnw[r * OL:(r + 1) * OL, None]),
            "lnb": np.ascontiguousarray(lnb[r * OL:(r + 1) * OL, None]),
        })
    return in_maps


def kernel(x, mask, centroids, conv_w, red_w, ln_w, ln_b, **run_kwargs):
    if "nc" not in _CACHE:
        _CACHE["nc"] = _build()
    nc = _CACHE["nc"]
    in_maps = _prep_inputs(x, mask, centroids, conv_w, red_w, ln_w, ln_b)
    res = run_bass_kernel_spmd(nc, in_maps, list(range(NCORES)), **run_kwargs)
    out = np.empty((N, OUT), dtype=np.float32)
    for r in range(NCORES):
        out[:, r * OL:(r + 1) * OL] = res.results[r]["yT"].T
    if run_kwargs:
        _CACHE["last_result"] = res
    return out


# revision 9
# speedup vs baseline: 1.5564x; 1.5564x over previous
"""NetVLAD Trainium2 kernel — 8-core SPMD Bass/Tile implementation.

Reference computation (N=32, C=512, T=300, R=4, K=64, OUT=1024):
  xf = x.reshape(N, C, T*R)
  logits = einsum('kc,nct->nkt', conv_w, xf); assign = softmax_k(logits)
  am = assign * mask;  vlad = einsum('nkt,nct->nkc', am, xf) - am.sum(t)*centroids
  vlad = l2norm_c(vlad).reshape(N, K*C); vlad = l2norm(vlad)
  y = vlad @ red_w.T;  out = layernorm(y) * ln_w + ln_b

Sharding: stage 1 data-parallel over N (4 samples/core, fp32);
bf16 AllGather of the per-sample normalized descriptors (global-norm folded in);
stage 2 output-parallel over OUT (128 cols/core) with bf16 weights;
AllReduce of LayerNorm partial sums; LN epilogue on device.

Host passes x in both c-major and t-major layouts so every matmul sees its
operands in natural orientation (no on-device transposes).
"""
import numpy as np
import ml_dtypes

import concourse.bass as bass
import concourse.bacc as bacc
import concourse.tile as tile
import concourse.mybir as mybir
from concourse.bass_utils import run_bass_kernel_spmd

NCORES = 8
N, C, T, R, K, OUT = 32, 512, 300, 4, 64, 1024
TR = T * R            # 1200
TRP = 1280            # padded to 10 chunks of 128 (pads masked out)
NJ = TRP // 128       # 10
NL = N // NCORES      # 4 samples per core
CC = C // 128         # 4 c-chunks
OL = OUT // NCORES    # 128 output cols per core
F32 = mybir.dt.float32
BF16 = mybir.dt.bfloat16
AX = mybir.AxisListType
AF = mybir.ActivationFunctionType

_CACHE = {}


def _build(debug=False):
    nc = bacc.Bacc("TRN2", target_bir_lowering=False, debug=False,
                   num_devices=NCORES)
    xc = nc.dram_tensor("xc", [NL, C, TRP], BF16, kind="ExternalInput")
    xt = nc.dram_tensor("xt", [NL, TRP, C], BF16, kind="ExternalInput")
    mf = nc.dram_tensor("mf", [NL, 128, NJ], F32, kind="ExternalInput")
    cwT_d = nc.dram_tensor("cwT", [C, K], BF16, kind="ExternalInput")
    cent_d = nc.dram_tensor("cent", [K, C], F32, kind="ExternalInput")
    id64_d = nc.dram_tensor("id64", [K, K], F32, kind="ExternalInput")
    w2_d = nc.dram_tensor("w2", [CC, 128, K * 128], BF16, kind="ExternalInput")
    lnw_d = nc.dram_tensor("lnw", [OL, 1], F32, kind="ExternalInput")
    lnb_d = nc.dram_tensor("lnb", [OL, 1], F32, kind="ExternalInput")
    yT_d = nc.dram_tensor("yT", [OL, N], F32, kind="ExternalOutput")
    if debug:
        dbg_vs_d = nc.dram_tensor("dbg_vs", [CC * 128, NL * K], BF16,
                                  kind="ExternalOutput")
        dbg_ys_d = nc.dram_tensor("dbg_ys", [OL, N], F32,
                                  kind="ExternalOutput")
        dbg_am_d = nc.dram_tensor("dbg_am", [TRP, K], F32,
                                  kind="ExternalOutput")
        dbg_vt_d = nc.dram_tensor("dbg_vt", [128, CC * K], F32,
                                  kind="ExternalOutput")
        dbg_sc_d = nc.dram_tensor("dbg_sc", [2, K], F32,
                                  kind="ExternalOutput")
    rg = [list(range(NCORES))]

    with tile.TileContext(nc) as tc:
        with tc.tile_pool(name="constp", bufs=1) as constp, \
             tc.tile_pool(name="xcp", bufs=8) as xcp, \
             tc.tile_pool(name="xtp", bufs=2 * NJ) as xtp, \
             tc.tile_pool(name="amp", bufs=2 * NJ) as amp, \
             tc.tile_pool(name="smp", bufs=4) as smp, \
             tc.tile_pool(name="vsp", bufs=2) as vsp, \
             tc.tile_pool(name="w2p", bufs=1) as w2p, \
             tc.tile_pool(name="vgp", bufs=1) as vgp, \
             tc.tile_pool(name="psA", bufs=2, space="PSUM") as psA, \
             tc.tile_pool(name="psV", bufs=2, space="PSUM") as psV, \
             tc.tile_pool(name="psS", bufs=2, space="PSUM") as psS, \
             tc.tile_pool(name="psY", bufs=1, space="PSUM") as psY, \
             tc.tile_pool(name="dram", bufs=1, space="DRAM") as dram:

            # ---- constants ----
            ones = constp.tile([128, 1], F32, name="ones")
            nc.vector.memset(ones[:], 1.0)
            onesb = constp.tile([128, 1], BF16, name="onesb")
            nc.vector.memset(onesb[:], 1.0)
            cwTt = []
            for cc in range(CC):
                cw = constp.tile([128, K], BF16, name=f"cwTt{cc}")
                nc.sync.dma_start(out=cw[:], in_=cwT_d[cc * 128:(cc + 1) * 128, :])
                cwTt.append(cw)
            centt = []
            for cc in range(CC):
                ce = constp.tile([K, 128], F32, name=f"centt{cc}")
                nc.sync.dma_start(out=ce[:], in_=cent_d[:, cc * 128:(cc + 1) * 128])
                centt.append(ce)
            id64t = constp.tile([K, K], F32, name="id64t")
            nc.sync.dma_start(out=id64t[:], in_=id64_d[:])
            mtn = []
            for n in range(NL):
                mt = constp.tile([128, NJ], F32, name=f"mt{n}")
                nc.sync.dma_start(out=mt[:], in_=mf[n])
                mtn.append(mt)
            lnw_t = constp.tile([OL, 1], F32, name="lnw_t")
            nc.sync.dma_start(out=lnw_t[:], in_=lnw_d[:])
            lnb_t = constp.tile([OL, 1], F32, name="lnb_t")
            nc.sync.dma_start(out=lnb_t[:], in_=lnb_d[:])

            cc_in = dram.tile([CC * 128, NL * K], BF16, name="cc_in")

            # ---- stage 1: per-sample NetVLAD descriptor ----
            for n in range(NL):
                xq = []
                for cc in range(CC):
                    xcq = xcp.tile([128, TRP], BF16, name="xcq")
                    nc.sync.dma_start(out=xcq[:],
                                      in_=xc[n, cc * 128:(cc + 1) * 128, :])
                    xq.append(xcq)
                xtq = []
                for jj in range(NJ):
                    xtt = xtp.tile([128, C], BF16, name="xtt")
                    nc.sync.dma_start(out=xtt[:],
                                      in_=xt[n, jj * 128:(jj + 1) * 128, :])
                    xtq.append(xtt)

                # logits (TR-major) + masked softmax over K
                amT = []
                for jj in range(NJ):
                    lt = psA.tile([128, K], F32, name="lt")
                    for cc in range(CC):
                        nc.tensor.matmul(
                            lt[:], lhsT=xq[cc][:, jj * 128:(jj + 1) * 128],
                            rhs=cwTt[cc][:],
                            start=(cc == 0), stop=(cc == CC - 1))
                    negmax = smp.tile([128, 1], F32, name="negmax")
                    nc.vector.reduce_max(negmax[:], lt[:], axis=AX.X, negate=True)
                    et = smp.tile([128, K], F32, name="et")
                    s = smp.tile([128, 1], F32, name="s")
                    nc.scalar.activation(et[:], lt[:], AF.Exp,
                                         bias=negmax[:], scale=1.0,
                                         accum_out=s[:])
                    rs = smp.tile([128, 1], F32, name="rs")
                    nc.vector.reciprocal(rs[:], s[:])
                    rsm = smp.tile([128, 1], F32, name="rsm")
                    nc.vector.tensor_mul(rsm[:], rs[:], mtn[n][:, jj:jj + 1])
                    am = amp.tile([128, K], BF16, name="am")
                    nc.vector.tensor_scalar_mul(am[:], et[:], rsm[:])
                    amT.append(am)
                    if debug and n == 0:
                        nc.sync.dma_start(
                            out=dbg_am_d[jj * 128:(jj + 1) * 128, :],
                            in_=am[:])

                # vlad^T accumulation: vt[c, cc*K+k] = sum_t x^T[t,c]*am[t,k]
                vt = psV.tile([128, CC * K], F32, name="vt")
                nc.vector.memset(vt[:], 0.0)
                for cc in range(CC):
                    for jj in range(NJ):
                        nc.tensor.matmul(
                            vt[:, cc * K:(cc + 1) * K],
                            lhsT=xtq[jj][:, cc * 128:(cc + 1) * 128],
                            rhs=amT[jj][:],
                            start=False, stop=False,
                            skip_group_check=True)
                # a[k] = sum_t am[t,k]
                a_ps = psS.tile([1, K], F32, name="a_ps", tag="small_ps")
                for jj in range(NJ):
                    nc.tensor.matmul(a_ps[:], lhsT=onesb[:], rhs=amT[jj][:],
                                     start=(jj == 0), stop=(jj == NJ - 1))
                na_sb = smp.tile([1, K], F32, name="na_sb")
                nc.scalar.mul(na_sb[:], a_ps[:], -1.0)
                naB = smp.tile([K, K], F32, name="naB")
                nc.gpsimd.partition_broadcast(naB[:], na_sb[:])
                dga = smp.tile([K, K], F32, name="dga")
                nc.vector.tensor_mul(dga[:], id64t[:], naB[:])
                # vt -= a[k]*cent[k,c]  (cent.T @ diag(a) appended to the group)
                for cc in range(CC):
                    nc.tensor.matmul(vt[:, cc * K:(cc + 1) * K],
                                     lhsT=centt[cc][:], rhs=dga[:],
                                     start=False, stop=(cc == CC - 1),
                                     skip_group_check=True)

                # intra-norm over c + global norm, fused into one scale
                if debug and n == 0:
                    vt_sb = smp.tile([128, CC * K], F32, name="vt_sb")
                    nc.scalar.copy(vt_sb[:], vt[:])
                    nc.sync.dma_start(out=dbg_vt_d[:], in_=vt_sb[:])
                vsq = smp.tile([128, CC * K], F32, name="vsq")
                nc.scalar.square(vsq[:], vt[:])
                ssq_ps = psS.tile([1, CC * K], F32, name="ssq_ps", tag="small_ps")
                nc.tensor.matmul(ssq_ps[:], lhsT=ones[:], rhs=vsq[:],
                                 start=True, stop=True)
                ssq = smp.tile([1, K], F32, name="ssq")
                nc.vector.reduce_sum(
                    ssq[:],
                    ssq_ps.rearrange("p (cc k) -> p k cc", cc=CC)[:],
                    axis=AX.X)
                nrm = smp.tile([1, K], F32, name="nrm")
                nc.scalar.sqrt(nrm[:], ssq[:])
                nrmc = smp.tile([1, K], F32, name="nrmc")
                nc.vector.tensor_scalar_max(nrmc[:], nrm[:], 1e-12)
                rn = smp.tile([1, K], F32, name="rn")
                nc.vector.reciprocal(rn[:], nrmc[:])
                t1 = smp.tile([1, K], F32, name="t1")
                nc.vector.tensor_mul(t1[:], ssq[:], rn[:])
                t2 = smp.tile([1, K], F32, name="t2")
                nc.vector.tensor_mul(t2[:], t1[:], rn[:])
                gsq = smp.tile([1, 1], F32, name="gsq")
                nc.vector.reduce_sum(gsq[:], t2[:], axis=AX.X)
                g = smp.tile([1, 1], F32, name="g")
                nc.scalar.sqrt(g[:], gsq[:])
                gc = smp.tile([1, 1], F32, name="gc")
                nc.vector.tensor_scalar_max(gc[:], g[:], 1e-12)
                gr = smp.tile([1, 1], F32, name="gr")
                nc.vector.reciprocal(gr[:], gc[:])
                rne = smp.tile([1, K], F32, name="rne")
                nc.vector.tensor_scalar_mul(rne[:], rn[:], gr[:])
                if debug and n == 0:
                    nc.sync.dma_start(out=dbg_sc_d[0:1, :], in_=ssq[:])
                    nc.sync.dma_start(out=dbg_sc_d[1:2, :], in_=rne[:])
                rnB = smp.tile([128, K], F32, name="rnB")
                nc.gpsimd.partition_broadcast(rnB[:], rne[:])
                vstar = vsp.tile([128, CC * K], BF16, name="vstar")
                for cc in range(CC):
                    nc.vector.tensor_mul(vstar[:, cc * K:(cc + 1) * K],
                                         vt[:, cc * K:(cc + 1) * K], rnB[:])
                nc.sync.dma_start(
                    out=cc_in.rearrange("(cc c) nk -> c cc nk",
                                        cc=CC)[:, :, n * K:(n + 1) * K],
                    in_=vstar.rearrange("c (cc k) -> c cc k", cc=CC)[:])

            if debug:
                nc.sync.dma_start(out=dbg_vs_d[:], in_=cc_in[:])

            # ---- AllGather descriptors ----
            cc_out = dram.tile([NCORES * CC * 128, NL * K], BF16,
                               addr_space="Shared", name="cc_out")
            nc.gpsimd.collective_compute(
                "AllGather", mybir.AluOpType.bypass, replica_groups=rg,
                ins=[cc_in.opt()], outs=[cc_out.opt()])

            # ---- stage 2: y^T[o, n] = sum_j v[n, j] * red_w[o, j] ----
            w2t = []
            for cc in range(CC):
                wt = w2p.tile([128, K * 128], BF16, name=f"w2t{cc}")
                nc.sync.dma_start(out=wt[:], in_=w2_d[cc])
                w2t.append(wt)
            vg = []
            for cc in range(CC):
                vgt = vgp.tile([128, NCORES * NL * K], BF16, name=f"vgt{cc}")
                nc.sync.dma_start(
                    out=vgt.rearrange("c (r nk) -> c r nk", r=NCORES)[:],
                    in_=cc_out.rearrange("(r cc c) nk -> cc c r nk",
                                         r=NCORES, cc=CC)[cc])
                vg.append(vgt)
            yT = psY.tile([OL, N], F32, name="yT")
            for cc in range(CC):
                vgv = vg[cc].rearrange("c (rn k) -> c k rn", k=K)
                for k in range(K):
                    nc.tensor.matmul(
                        yT[:], lhsT=w2t[cc][:, k * 128:(k + 1) * 128],
                        rhs=vgv[:, k],
                        start=(cc == 0 and k == 0),
                        stop=(cc == CC - 1 and k == K - 1))

            # ---- LayerNorm: stats over all OUT via AllReduce ----
            ys = smp.tile([OL, N], F32, name="ys")
            nc.scalar.copy(ys[:], yT[:])
            if debug:
                nc.sync.dma_start(out=dbg_ys_d[:], in_=ys[:])
            ysq = smp.tile([OL, N], F32, name="ysq")
            nc.vector.tensor_mul(ysq[:], ys[:], ys[:])
            st_ps = psS.tile([1, 2 * N], F32, name="st_ps", tag="small_ps")
            nc.vector.memset(st_ps[:], 0.0)
            nc.tensor.matmul(st_ps[:, :N], lhsT=ones[:], rhs=ys[:],
                             start=False, stop=True, skip_group_check=True)
            nc.tensor.matmul(st_ps[:, N:], lhsT=ones[:], rhs=ysq[:],
                             start=False, stop=True, skip_group_check=True)
            st_sb = smp.tile([1, 2 * N], F32, name="st_sb")
            nc.scalar.copy(st_sb[:], st_ps[:])
            ar_in = dram.tile([1, 2 * N], F32, name="ar_in")
            nc.sync.dma_start(out=ar_in[:], in_=st_sb[:])
            ar_out = dram.tile([1, 2 * N], F32, addr_space="Shared",
                               name="ar_out")
            nc.gpsimd.collective_compute(
                "AllReduce", mybir.AluOpType.add, replica_groups=rg,
                ins=[ar_in.opt()], outs=[ar_out.opt()])
            st = smp.tile([1, 2 * N], F32, name="st")
            nc.sync.dma_start(out=st[:], in_=ar_out[:])
            mu = smp.tile([1, N], F32, name="mu")
            nc.vector.tensor_scalar_mul(mu[:], st[:, :N], 1.0 / OUT)
            m2 = smp.tile([1, N], F32, name="m2")
            nc.vector.tensor_scalar_mul(m2[:], st[:, N:], 1.0 / OUT)
            musq = smp.tile([1, N], F32, name="musq")
            nc.vector.tensor_mul(musq[:], mu[:], mu[:])
            var = smp.tile([1, N], F32, name="var")
            nc.vector.tensor_sub(var[:], m2[:], musq[:])
            vare = smp.tile([1, N], F32, name="vare")
            nc.vector.tensor_scalar_add(vare[:], var[:], 1e-5)
            sd = smp.tile([1, N], F32, name="sd")
            nc.scalar.sqrt(sd[:], vare[:])
            rstd = smp.tile([1, N], F32, name="rstd")
            nc.vector.reciprocal(rstd[:], sd[:])
            muB = smp.tile([OL, N], F32, name="muB")
            nc.gpsimd.partition_broadcast(muB[:], mu[:])
            rstdB = smp.tile([OL, N], F32, name="rstdB")
            nc.gpsimd.partition_broadcast(rstdB[:], rstd[:])
            d1 = smp.tile([OL, N], F32, name="d1")
            nc.vector.tensor_sub(d1[:], ys[:], muB[:])
            d2 = smp.tile([OL, N], F32, name="d2")
            nc.vector.tensor_mul(d2[:], d1[:], rstdB[:])
            outv = smp.tile([OL, N], F32, name="outv")
            nc.scalar.activation(outv[:], d2[:], AF.Identity,
                                 bias=lnb_t[:], scale=lnw_t[:])
            nc.sync.dma_start(out=yT_d[:], in_=outv[:])

    nc.compile()
    return nc


def _prep_inputs(x, mask, centroids, conv_w, red_w, ln_w, ln_b):
    """Shard + lay out the full inputs per core."""
    x = np.asarray(x, dtype=np.float32)
    mask = np.asarray(mask)
    xf = x.reshape(N, C, TR)
    xc_full = np.zeros((N, C, TRP), dtype=np.float32)
    xc_full[:, :, :TR] = xf
    xt_full = np.ascontiguousarray(xc_full.transpose(0, 2, 1))
    m_e = np.zeros((N, TRP), dtype=np.float32)
    m_e[:, :TR] = np.repeat(np.asarray(mask, dtype=np.float32), R, axis=1)
    # mf[n, p, jj] = m_e[n, jj*128 + p]
    mf_full = np.ascontiguousarray(
        m_e.reshape(N, NJ, 128).transpose(0, 2, 1))
    cwT = np.ascontiguousarray(
        np.asarray(conv_w, dtype=np.float32).T).astype(ml_dtypes.bfloat16)
    cent = np.ascontiguousarray(np.asarray(centroids, dtype=np.float32))
    id64 = np.eye(K, dtype=np.float32)
    rw = np.asarray(red_w, dtype=np.float32)
    lnw = np.asarray(ln_w, dtype=np.float32)
    lnb = np.asarray(ln_b, dtype=np.float32)

    in_maps = []
    for r in range(NCORES):
        ns = slice(r * NL, (r + 1) * NL)
        # w2[cc][c, k*128+o] = red_w[r*128+o, k*512+cc*128+c]
        wl = rw[r * OL:(r + 1) * OL].reshape(OL, K, CC, 128)  # [o,k,cc,c]
        w2 = np.stack(
            [np.ascontiguousarray(
                wl[:, :, cc, :].transpose(2, 1, 0).reshape(128, K * 128))
             for cc in range(CC)], axis=0).astype(ml_dtypes.bfloat16)
        in_maps.append({
            "xc": np.ascontiguousarray(xc_full[ns]).astype(ml_dtypes.bfloat16),
            "xt": np.ascontiguousarray(xt_full[ns]).astype(ml_dtypes.bfloat16),
            "mf": np.ascontiguousarray(mf_full[ns]),
            "cwT": cwT,
            "cent": cent,
            "id64": id64,
            "w2": w2,
            "lnw": np.ascontiguousarray(lnw[r * OL:(r + 1) * OL, None]),
            "lnb": np.ascontiguousarray(lnb[r * OL:(r + 1) * OL, None]),
        })
    return in_maps


def kernel(x, mask, centroids, conv_w, red_w, ln_w, ln_b, **run_kwargs):
    if "nc" not in _CACHE:
        _CACHE["nc"] = _build()
    nc = _CACHE["nc"]
    in_maps = _prep_inputs(x, mask, centroids, conv_w, red_w, ln_w, ln_b)
    res = run_bass_kernel_spmd(nc, in_maps, list(range(NCORES)), **run_kwargs)
    out = np.empty((N, OUT), dtype=np.float32)
    for r in range(NCORES):
        out[:, r * OL:(r + 1) * OL] = res.results[r]["yT"].T
    if run_kwargs:
        _CACHE["last_result"] = res
    return out
